# revision 45
# baseline (speedup 1.0000x reference)
"""Causal single-head attention (b=4, n=2048, d=1024, fp32) on 8 TRN2 NeuronCores.

Sharding v4 — uniform padded zig-zag q-split. Core c = (batch c//2, role c%2).
Each role owns 8 of the 16 query subtiles of its batch (zig-zag interleaved,
see ROLE_SUBTILES); every core produces out rows for its own 1024 queries
with the FULL 1024 features.

The SPMD program is identical on all cores; the role only changes host-side
data: which columns land in xq (own queries), the mask flags, and where host
scatters the output rows. Causal work is padded to the per-position envelope
across the two roles (AV_ENV) at 128-query granularity; mask data zeroes the
padding: per (slot, pos) only the last two k-chunks differ between roles
(interior chunks are all-ones for both), so masks are built ON-CHIP from a
shared [128,128] triangle and a per-role 0/1 flag per position.

Per core pipeline:
  Scores use the algebraic fold S = x (W_q^T W_k) x^T: the host precomputes
  M = W_q^T W_k in fp32 and ships it e4m3; the kernel computes
  zq = M^T xq^T and the scores S = zq^T x^T both as fp8-e4m3 DoubleRow
  matmuls (2x PE rate, contraction pairs of 128-row d-chunks). End-to-end
  rel err 1.75e-2 (gate 2e-2, deterministic on the fixed seed-0 inputs);
  fp8 on the AV or out matmuls would blow the gate (measured 3.5e-2+), so
  those stay bf16.
  P = exp(S/(32*512)) * mask (no max subtraction; true scaled scores are in
  [-2.6, 2.6]); row sums accumulate on the DVE across k-chunks, finished by
  one ones-matmul per 128-query block. U^T[b,q] = x^T P rides one
  width-narrowing accumulation chain per feature chunk (the active causal
  positions are a contiguous q-suffix per k-chunk, so score and U matmuls
  cover them in single wide instructions), and out[q,o] = (U W_v^T)/l.

DMA: the zq-critical m/xq stream (2MB fp8) owns the sync HWDGE ring in
consumption order (a ring's transfers drain FIFO; a second active queue
would halve the critical arrival rate). The attention-phase bulk
(xT/xN/wv, 8MB) rides the gpsimd SWDGE ring, each piece gated by a 1-col
dep-copy on the tail of the m stream so its transfers never contend with
the ramp. Output is written bf16 and widened on host.
"""

import os
import sys

if os.path.isdir("/opt/trn_rl_repo") and "/opt/trn_rl_repo" not in sys.path:
    sys.path.insert(0, "/opt/trn_rl_repo")

import numpy as np
import ml_dtypes

BF16 = ml_dtypes.bfloat16
F8E4 = ml_dtypes.float8_e4m3  # TRN fp8_e4: bias 7, max +-240, has inf

B, N, D = 4, 2048, 1024
NCORES = 8
P = 128
QT = 512
NKC = N // P       # 16 k chunks
NDC = D // P       # 8 d chunks
NPR = NDC // 2     # 4 d-chunk PAIRS (fp8 DoubleRow contraction granularity)
NQ_OWN = 1024      # own queries per core
SCALE = 1.0 / 32.0
# fp8 scales for the S = zq . x^T matmul (e4m3 max +-240; zq std ~0.33,
# x std 1 -> scaled operands stay well inside the normal range)
ZQ_SC = 16.0
X_SC = 32.0
M_SC = 1024.0
S_DESC = 1.0 / (ZQ_SC * X_SC)

# Zig-zag assignment of the 16 query subtiles (128 rows each) to the two
# roles, chosen so the per-position envelope across roles is minimal.
ROLE_SUBTILES = {
    0: (0, 3, 4, 7, 8, 11, 12, 15),
    1: (1, 2, 5, 6, 9, 10, 13, 14),
}
# k-chunk envelope per (slot, position): max over both roles of the
# causally-needed chunk count for the subtile each role places there.
AV_ENV = ((2, 4, 6, 8), (10, 12, 14, 16))

_CACHE = {}


def _build_module():
    from concourse import bacc
    import concourse.tile as tile
    import concourse.mybir as mybir

    bf = mybir.dt.bfloat16
    f8 = mybir.dt.float8e4
    f32 = mybir.dt.float32
    Exp = mybir.ActivationFunctionType.Exp
    DR = mybir.MatmulPerfMode.DoubleRow

    nc = bacc.Bacc("TRN2", target_bir_lowering=False, debug=False, num_devices=NCORES)

    # All inputs partition-major: [128, ...] with multi-KiB contiguous runs.
    tri_d = nc.dram_tensor("tri", [P, P], bf, kind="ExternalInput")
    flg_d = nc.dram_tensor("flg", [P, 8], f32, kind="ExternalInput")
    m_d = nc.dram_tensor("m", [P, NDC * NDC * P], f8, kind="ExternalInput")
    xq_d = nc.dram_tensor("xq", [P, 2 * NDC * QT], f8, kind="ExternalInput")
    # x^T for the scores contraction ships as e4m3 (x * 32), packed in
    # DoubleRow pair layout per k-chunk [p, kc, pr, j, k]: feature
    # d = (2*pr + j)*128 + p, key n = kc*128 + k.
    xT_d = nc.dram_tensor("xT", [P, NKC * NPR * 2 * P], f8, kind="ExternalInput")
    xN_d = nc.dram_tensor("xN", [P, NKC * D], bf, kind="ExternalInput")
    wv_d = nc.dram_tensor("wv", [P, NDC * D], bf, kind="ExternalInput")
    out_d = nc.dram_tensor("out", [NQ_OWN, D], bf, kind="ExternalOutput")

    out_r = out_d.ap().rearrange("(s p) o -> p s o", p=P)

    with tile.TileContext(nc) as tc:
        with tc.tile_pool(name="pers", bufs=1) as pers:
            zq = pers.tile([P, NPR, 2, NQ_OWN], f8, tag="zq")
            xT = pers.tile([P, NKC, NPR, 2, P], f8, tag="xT")
            xN = pers.tile([P, NKC, D], bf, tag="xN")
            wv = pers.tile([P, 2, NDC, QT], bf, tag="wv")
            tri = pers.tile([P, P], bf, tag="tri")
            flg = pers.tile([P, 8], f32, tag="flg")
            msk = pers.tile([P, 2, 4, 2, P], bf, tag="msk")
            ones = pers.tile([P, 1], bf, tag="ones")

            nc.vector.memset(ones[:], 1.0)

            # ---- zq projection (fp8 DoubleRow, like the scores) ----
            with (
                tc.tile_pool(name="wp", bufs=1) as wp,
                tc.tile_pool(name="xsp", bufs=1) as xsp,
                tc.tile_pool(name="psA", bufs=8, space="PSUM") as psA,
            ):
                # m packed e4m3 (x M_SC) in DoubleRow pair layout
                # [p, dch, g, pq, b2, j2, j]: d_in = (dch*4 + pq*2 + j2)*128
                # + p, d_out = (2g + b2)*128 + j. The zq loop runs the dc
                # halves OUTERMOST across all 4 groups so the m/xq arrival
                # demand is smooth instead of a burst for the first group.
                m = wp.tile([P, 2, NPR, 2, 2, 2, P], f8, tag="m")
                # xq packed e4m3 (x X_SC) [p, qt, dcp, j2, q]
                xq = xsp.tile([P, 2, NPR, 2, QT], f8, tag="xq")
                # PE pre-warm while the first DMAs land (HAM ramp). memset on
                # gpsimd: it exits the runtime preamble ~1.5us before DVE, so
                # the warm matmuls can fire the moment PE's queue opens.
                wsrc = pers.tile([P, QT], bf, tag="wsrc")
                nc.gpsimd.memset(wsrc[:], 0.0)
                wps = psA.tile([P, QT], f32, tag="proj", name="warm")
                for _ in range(7):
                    nc.tensor.matmul(wps, wsrc[:, :P], wsrc[:], start=True, stop=True)

                m_f = m[:].rearrange("p h g pq b2 j2 j -> p (h g pq b2 j2 j)")
                xq_f = xq[:].rearrange("p qt dcp j2 q -> p (qt dcp j2 q)")
                # The critical stream rides TWO independent FIFO queues --
                # m on the sync ring, xq on the scalar ring (idle until the
                # zq casts at ~14us) -- so the first-chunk arrival ladders
                # run in parallel and one slow chunk can't stall the other
                # stream. Each stream only needs ~75GB/s, well under a
                # contended ring's share.
                qact = pers.tile([P, 1], bf, tag="qact")
                nc.sync.dma_start(qact[:], tri_d.ap()[:, 0:1])
                nc.scalar.dma_start(xq_f[:, 0:1024], xq_d.ap()[:, 0:1024])
                nc.sync.dma_start(m_f[:, 0:1024], m_d.ap()[:, 0:1024])
                nc.scalar.dma_start(xq_f[:, 1024:2048], xq_d.ap()[:, 1024:2048])
                nc.sync.dma_start(m_f[:, 1024:2048], m_d.ap()[:, 1024:2048])
                nc.sync.dma_start(m_f[:, 2048:4096], m_d.ap()[:, 2048:4096])
                nc.scalar.dma_start(xq_f[:, 2048:4096], xq_d.ap()[:, 2048:4096])
                nc.sync.dma_start(m_f[:, 4096:6144], m_d.ap()[:, 4096:6144])
                nc.sync.dma_start(m_f[:, 6144:8192], m_d.ap()[:, 6144:8192])
                nc.scalar.dma_start(xq_f[:, 4096:8192], xq_d.ap()[:, 4096:8192])
                xT_f = xT[:].rearrange("p kc pr j k -> p (kc pr j k)")
                XTC = NPR * 2 * P  # 1024 fp8 elems per k-chunk per partition
                xN_f = xN[:].rearrange("p kc b -> p (kc b)")
                wv_f = wv[:].rearrange("p oh dc o -> p (oh dc o)")
                # Bulk attention-phase inputs ride the gpsimd (SWDGE) ring,
                # each gated by a 1-col dep-copy reading the tail of the m
                # stream: the issue can't start until the zq-critical sync
                # queue has mostly drained, so the bulk never steals DMA
                # bandwidth from the ramp. The copied bytes are dummy values
                # overwritten by the bulk DMA itself (WAW ordering). Order =
                # consumption order of the attention phase.
                bf16_t = mybir.dt.bfloat16
                mdep = m[:, 1, NPR - 1, 1, 1, 1, P - 2 : P].bitcast(bf16_t)
                HX = NKC * D // 2
                bulk = [
                    (tri[:, 0:1], tri[:], tri_d.ap()),
                    (flg[:, 0:1], flg[:], flg_d.ap()),
                    (xT[:, 0, 0, 0, 0:2].bitcast(bf16_t),
                     xT_f[:, : 8 * XTC], xT_d.ap()[:, : 8 * XTC]),
                    (xN[:, 0, 0:1], xN_f[:, : HX // 2], xN_d.ap()[:, : HX // 2]),
                    (wv[:, 0, 0, 0:1], wv_f[:, :4096], wv_d.ap()[:, :4096]),
                    (wv[:, 1, 0, 0:1], wv_f[:, 4096:], wv_d.ap()[:, 4096:]),
                    (xN[:, 4, 0:1],
                     xN_f[:, HX // 2 : HX], xN_d.ap()[:, HX // 2 : HX]),
                    (xT[:, 8, 0, 0, 0:2].bitcast(bf16_t),
                     xT_f[:, 8 * XTC :], xT_d.ap()[:, 8 * XTC :]),
                    (xN[:, 8, 0:1], xN_f[:, HX:], xN_d.ap()[:, HX:]),
                ]
                for dep_dst, dst, src in bulk:
                    nc.gpsimd.tensor_copy(dep_dst, mdep)
                    nc.gpsimd.dma_start(dst, src)

                # On-chip mask build: per (slot, pos), chunk env-2 is
                # max(tri, flag) (all-ones for the bigger role, triangle for
                # the smaller) and chunk env-1 is tri*flag (triangle for the
                # bigger role, zeros for the smaller). flag = 1 iff this
                # role's subtile at the position is the bigger one.
                for slot in range(2):
                    for pos in range(4):
                        fcol = flg[:, slot * 4 + pos : slot * 4 + pos + 1]
                        nc.vector.tensor_scalar_max(
                            msk[:, slot, pos, 0, :], tri[:], fcol
                        )
                        nc.vector.tensor_scalar_mul(
                            msk[:, slot, pos, 1, :], tri[:], fcol
                        )

                # zq projection: zq[b, q] = M^T xq^T (own 1024 q). All 8
                # PSUM groups of a qt-half accumulate together, dc-half
                # outermost; the casts ride the second dc-half per group so
                # the next qt's bank reuse never stalls the PE.
                for qt in range(2):
                    pss = [
                        psA.tile([P, QT], f32, tag="proj", name=f"pj{qt}{i}")
                        for i in range(8)
                    ]
                    for g in range(NPR):
                        for pq in range(2):
                            for b2 in range(2):
                                nc.tensor.matmul(
                                    pss[g * 2 + b2],
                                    m[:, 0, g, pq, b2, :, :],
                                    xq[:, qt, pq, :, :],
                                    start=(pq == 0),
                                    stop=False,
                                    perf_mode=DR,
                                )
                    for g in range(NPR):
                        for pq in range(2):
                            for b2 in range(2):
                                nc.tensor.matmul(
                                    pss[g * 2 + b2],
                                    m[:, 1, g, pq, b2, :, :],
                                    xq[:, qt, 2 + pq, :, :],
                                    start=False,
                                    stop=(pq == 1),
                                    perf_mode=DR,
                                )
                        # scalar-engine cast to e4m3 (x ZQ_SC): the DVE is the
                        # attention phase's busy engine; zq casts queued there
                        # delay slot-1 scores behind mask-muls and uT casts.
                        for b2 in range(2):
                            nc.scalar.mul(
                                zq[:, g, b2, qt * QT : (qt + 1) * QT],
                                pss[g * 2 + b2], ZQ_SC / (M_SC * X_SC),
                            )

            # ---- attention ----
            with (
                tc.tile_pool(name="stps", bufs=3, space="PSUM") as stps,
                tc.tile_pool(name="smps", bufs=1, space="PSUM") as smps,
                tc.tile_pool(name="psU", bufs=2, space="PSUM") as psU,
                tc.tile_pool(name="outp", bufs=2, space="PSUM") as outp,
                tc.tile_pool(name="pTp", bufs=2) as pTp,
                tc.tile_pool(name="uTp", bufs=1) as uTp,
                tc.tile_pool(name="rap", bufs=2) as rap,
                tc.tile_pool(name="outst", bufs=2) as outst,
                tc.tile_pool(name="rcpp", bufs=8) as rcpp,
            ):
                uT = uTp.tile([P, NDC, NQ_OWN], bf, tag="uT")
                for slot in range(2):
                    env = AV_ENV[slot]
                    nk3 = env[3]
                    # pos_lo[c]: first position whose causal envelope still
                    # includes k-chunk c -- chunk matmuls cover the q-column
                    # range [pos_lo*128, 512) in ONE instruction (the active
                    # positions are always a contiguous suffix).
                    pos_lo = [next(p for p in range(4) if env[p] > c)
                              for c in range(nk3)]
                    sheet = pTp.tile([P, NKC, QT], bf, tag="sheet")
                    racc = rap.tile([P, QT], bf, tag="racc")
                    for c in range(nk3):
                        w = slice(pos_lo[c] * P, QT)
                        zqs = slice(slot * QT + pos_lo[c] * P, (slot + 1) * QT)
                        ps = stps.tile([P, QT], f32, tag="st")
                        pw = ps[:, w]
                        for pr in range(NPR):
                            nc.tensor.matmul(
                                pw,
                                xT[:, c, pr, :, :],
                                zq[:, pr, :, zqs],
                                start=(pr == 0),
                                stop=(pr == NPR - 1),
                                perf_mode=DR,
                            )
                        nc.scalar.activation(
                            sheet[:, c, w], pw, Exp, bias=0.0,
                            scale=SCALE * S_DESC,
                        )
                        for pos in range(pos_lo[c], 4):
                            if c == env[pos] - 2:
                                nc.vector.tensor_mul(
                                    sheet[:, c, pos * P : (pos + 1) * P],
                                    sheet[:, c, pos * P : (pos + 1) * P],
                                    msk[:, slot, pos, 0, :],
                                )
                            elif c == env[pos] - 1:
                                nc.vector.tensor_mul(
                                    sheet[:, c, pos * P : (pos + 1) * P],
                                    sheet[:, c, pos * P : (pos + 1) * P],
                                    msk[:, slot, pos, 1, :],
                                )
                        if c == 0:
                            nc.vector.tensor_copy(racc[:], sheet[:, 0, :])
                        else:
                            nc.vector.tensor_add(
                                racc[:, w], racc[:, w], sheet[:, c, w]
                            )
                    # row sums + reciprocals first: racc is final once the
                    # S phase ends, and r is ready well before the out
                    # chains need it.
                    rs_r = []
                    for pos in range(4):
                        sm = smps.tile([P, 1], f32, tag="sm")
                        nc.tensor.matmul(
                            sm, racc[:, pos * P : (pos + 1) * P], ones[:],
                            start=True, stop=True,
                        )
                        r = rcpp.tile([P, 1], f32, tag="rcp", name=f"r{slot}{pos}")
                        nc.vector.reciprocal(r[:], sm)
                        rs_r.append(r)
                    # U^T[b, q] = x^T P: one width-narrowing accumulation
                    # chain per feature chunk bt covering the whole slot.
                    for bt in range(NDC):
                        ps = psU.tile([P, QT], f32, tag="ut")
                        for c in range(nk3):
                            nc.tensor.matmul(
                                ps[:, pos_lo[c] * P :],
                                xN[:, c, bt * P : (bt + 1) * P],
                                sheet[:, c, pos_lo[c] * P :],
                                start=(c == 0),
                                stop=(c == nk3 - 1),
                                skip_group_check=True,
                            )
                        udst = uT[:, bt, slot * QT : (slot + 1) * QT]
                        if bt % 2 == 0:
                            nc.vector.tensor_copy(udst, ps)
                        else:
                            nc.scalar.mul(udst, ps, 1.0)
                    for pos in range(4):
                        r = rs_r[pos]
                        # out[q, o] = U Wv^T, normalized by the row sums
                        s_idx = slot * 4 + pos
                        qs2 = slice(slot * QT + pos * P, slot * QT + (pos + 1) * P)
                        ot = outst.tile([P, D], bf, tag="ot")
                        last = slot == 1 and pos == 3
                        if not last:
                            for oh in range(2):
                                ps = outp.tile([P, QT], f32, tag="out")
                                for bc in range(NDC):
                                    nc.tensor.matmul(
                                        ps,
                                        uT[:, bc, qs2],
                                        wv[:, oh, bc, :],
                                        start=(bc == 0),
                                        stop=(bc == NDC - 1),
                                    )
                                if oh == 0:
                                    nc.scalar.mul(ot[:, :QT], ps, r[:])
                                    nc.sync.dma_start(
                                        out_r[:, s_idx, :QT], ot[:, :QT]
                                    )
                                else:
                                    nc.vector.tensor_scalar_mul(
                                        ot[:, QT:], ps, r[:]
                                    )
                                    nc.sync.dma_start(
                                        out_r[:, s_idx, QT:], ot[:, QT:]
                                    )
                        else:
                            # final block: 4 separate 256-col accumulation
                            # chains, each scaled and shipped as soon as its
                            # chain stops, so the tail's last DMA is small
                            # and the store drain overlaps the final MMs.
                            bounds = (0, 256, 512, 768, 896, 1024)
                            for piece in range(5):
                                lo, hi = bounds[piece], bounds[piece + 1]
                                oh, o0 = lo // QT, lo % QT
                                pw = hi - lo
                                psf = outp.tile([P, QT], f32, tag="out")
                                ps = psf[:, :pw]
                                for bc in range(NDC):
                                    nc.tensor.matmul(
                                        ps,
                                        uT[:, bc, qs2],
                                        wv[:, oh, bc, o0 : o0 + pw],
                                        start=(bc == 0),
                                        stop=(bc == NDC - 1),
                                    )
                                cs = slice(lo, hi)
                                if piece % 2 == 0:
                                    nc.scalar.mul(ot[:, cs], ps, r[:])
                                    nc.scalar.dma_start(
                                        out_r[:, s_idx, cs], ot[:, cs]
                                    )
                                else:
                                    nc.vector.tensor_scalar_mul(
                                        ot[:, cs], ps, r[:]
                                    )
                                    nc.sync.dma_start(
                                        out_r[:, s_idx, cs], ot[:, cs]
                                    )

    nc.compile()
    return nc


def _pack_pm(a):
    """[G*128, C] row-major -> [128, G*C] partition-major (2KB+ runs)."""
    g = a.shape[0] // P
    return np.ascontiguousarray(
        a.reshape(g, P, -1).transpose(1, 0, 2).reshape(P, -1)
    )


def get_module():
    if "nc" not in _CACHE:
        _CACHE["nc"] = _build_module()
    return _CACHE["nc"]


def make_in_maps(x, W_q, W_k, W_v):
    x = np.asarray(x, dtype=np.float32)
    W_q = np.asarray(W_q, dtype=np.float32)
    W_k = np.asarray(W_k, dtype=np.float32)
    # scores fold: S = x (W_q^T W_k) x^T -- M computed once in fp32,
    # shipped e4m3 (x M_SC) in DoubleRow pair layout
    # [p, dch, g, pq, b2, j2, j] = M[(dch*4+pq*2+j2)*128+p, (2g+b2)*128+j]
    m = (W_q.T.astype(np.float64) @ W_k.astype(np.float64) * M_SC).astype(F8E4)
    m_p = np.ascontiguousarray(
        m.reshape(2, 2, 2, P, NPR, 2, P)
        .transpose(3, 0, 4, 1, 5, 2, 6).reshape(P, -1)
    )
    wvT = np.asarray(W_v, dtype=np.float32).T.astype(BF16)  # [d_in, o]
    # wv packed [p, oh, dc, o'] = wvT[dc*128+p, oh*512+o']
    wv_p = np.ascontiguousarray(
        wvT.reshape(NDC, P, 2, QT).transpose(1, 2, 0, 3).reshape(P, -1)
    )
    k = np.arange(P)
    tri = (k[:, None] <= k[None, :]).astype(BF16)  # tri[p, q] = p <= q
    flgs = []
    for role in range(2):
        f = np.zeros((P, 8), dtype=np.float32)
        for slot in range(2):
            for pos in range(4):
                env = AV_ENV[slot][pos]
                nk_r = ROLE_SUBTILES[role][slot * 4 + pos] + 1
                f[:, slot * 4 + pos] = 1.0 if nk_r == env else 0.0
        flgs.append(f)
    in_maps = []
    for bidx in range(B):
        xb = x[bidx].astype(BF16)             # [N, D]
        # fp8 x^T in DoubleRow pair layout [p, kc, pr, j, k]:
        # value = x[kc*128+k, (2*pr+j)*128+p] * X_SC as e4m3
        x8 = (x[bidx] * X_SC).astype(F8E4)    # [N, D] from fp32
        xT_p = np.ascontiguousarray(
            x8.reshape(NKC, P, NPR, 2, P).transpose(4, 0, 2, 3, 1).reshape(P, -1)
        )
        xN_p = _pack_pm(xb)
        for r in range(2):
            qg = np.concatenate(
                [np.arange(s * P, (s + 1) * P) for s in ROLE_SUBTILES[r]]
            )
            xqsel = (x[bidx][qg].T * X_SC).astype(F8E4)   # [D, 1024] e4m3
            # xq packed [p, qt, dcp, j2, q] = xqsel[(dcp*2+j2)*128+p, qt*512+q]
            xq_p = np.ascontiguousarray(
                xqsel.reshape(NPR, 2, P, 2, QT)
                .transpose(2, 3, 0, 1, 4).reshape(P, -1)
            )
            in_maps.append(
                {
                    "tri": tri,
                    "flg": flgs[r],
                    "m": m_p,
                    "xq": xq_p,
                    "xT": xT_p,
                    "xN": xN_p,
                    "wv": wv_p,
                }
            )
    return in_maps


def kernel(x, W_q, W_k, W_v):
    from concourse.bass_utils import run_bass_kernel_spmd

    nc = get_module()
    in_maps = make_in_maps(x, W_q, W_k, W_v)
    res = run_bass_kernel_spmd(
        nc,
        in_maps,
        list(range(NCORES)),
        trace=bool(int(os.environ.get("KERNEL_TRACE", "0"))),
    )
    _CACHE["last_result"] = res
    out = np.empty((B, N, D), dtype=np.float32)
    for c in range(NCORES):
        b, r = c // 2, c % 2
        res_out = np.asarray(res.results[c]["out"]).astype(np.float32)
        for i, s in enumerate(ROLE_SUBTILES[r]):
            out[b, s * P : (s + 1) * P, :] = res_out[i * P : (i + 1) * P]
    return out
